# revision 36
# baseline (speedup 1.0000x reference)
"""Trainium2 Bass kernel for nn_NoiseProjector.

Strategy (8 NeuronCores):
- Data-parallel conv trunk: each core runs conv1+conv2+GAP on 8 of 64 images.
- Tiny AllGather of the pooled features (8x64 -> 64x64 per core).
- Tensor-parallel FC stage: fc_third (w3) / fc_cov (wc) / fc_mean (wm) weights
  column-sharded over output_dim (128 outputs per core).  The third-order term
  is computed as  T1 = cov @ W3r.T  (W3r = w3 reshaped (OUT*64, 4096)) followed
  by a small contraction over i:  third[b,o] = sum_i fc[b,i] * T1[b, o*64+i].
- w3 is pre-transposed host-side to K-major layout and cast to fp8e4m3 (wc to
  fp16) so the dominant 1 GiB weight stream is clean contiguous DMA at 1/4 the
  bytes; measured end-to-end relative error ~1.3e-3.
- The conv1 input is pre-tiled on the host into the exact (dy,ci)-replicated
  SBUF layout (one contiguous DMA per y-chunk instead of 336 small strided
  DMAs, which cost ~0.7 us each).
- Host concatenates the 8 per-core (64,128) outputs into (64,1024).
"""

import sys

sys.path.insert(0, "/opt/trn_rl_repo")

import numpy as np
import ml_dtypes

B = 64          # global batch
BL = 8          # images per core
NCORES = 8
OPC = 128       # outputs per core
FEAT = 64
JK = 4096       # FEAT*FEAT contraction
H, W = 224, 224
H1, W1 = 112, 112   # conv1 out
H2, W2 = 56, 56     # conv2 out
YC_HOST = 8         # conv1 y-rows per chunk (host pre-tiled layout)
GAP = 1.0 / (H2 * W2)

# --- symmetric-fold third-order stage tables -------------------------------
# pairs (j,k) j<=k sorted descending by j; K-chunks of 128, DoubleRow pairs
# of chunks (256 pairs per "g" group), 9 groups (2304 slots, 2080 real).
NG = 9                      # DoubleRow K-groups
NCH = 2 * NG                # 18 K-chunks of 128
NPAIR_PAD = NCH * 128       # 2304
_PAIRS = [(j, k) for j in range(63, -1, -1) for k in range(j, 64)]
NPAIR = len(_PAIRS)         # 2080
_PAIRS_P = _PAIRS + [(0, 0)] * (NPAIR_PAD - NPAIR)
# blocks of 4 i-values x 128 outputs (512 cols) active per group
NBG = []
for _g in range(NG):
    _jmax = max(j for j, _ in _PAIRS[_g * 256:min((_g + 1) * 256, NPAIR)])
    NBG.append(_jmax // 4 + 1)
NBLK = sum(NBG)             # 58 stream tiles
TOTC = NBLK * 1024          # stream columns (fp8 bytes per partition row)
# gmax[b] = last group contributing to i-block b
GMAX = [max(g for g in range(NG) if NBG[g] >= b + 1) for b in range(16)]
NWCH = 17                   # chunks carrying real pairs (2080 <= 17*128)
NBLK_A = 29                 # stream split: blocks in the first big DMA



def _split_multiwait_json(raw):
    """This walrus build accepts only ONE sync wait per instruction.  Split any
    multi-wait instruction into single-wait EventSemaphore ops ahead of it (the
    engine is in-order, so chained waits are equivalent)."""
    import json

    j = json.loads(raw)
    n_split = 0
    for f in j["functions"]:
        for bb in f["blocks"]:
            insts = bb.get("instructions")
            if not insts:
                continue
            out = []
            changed = False
            for ins in insts:
                si = ins.get("sync_info")
                waits = si.get("on_wait") if si else None
                if waits and len(waits) > 1:
                    changed = True
                    keep = None
                    for w in waits:
                        if w.get("wait_reg") is not None:
                            keep = w
                    if keep is None:
                        keep = waits[-1]
                    rest = [w for w in waits if w is not keep]
                    for k, w in enumerate(rest):
                        n_split += 1
                        out.append({
                            "engine": ins["engine"], "ins": [], "outs": [],
                            "name": f"{ins['name']}-sw{k}",
                            "opcode": "EventSemaphore",
                            "sync_info": {"on_update": [], "on_wait": [w]},
                        })
                    si["on_wait"] = [keep]
                out.append(ins)
            if changed:
                bb["instructions"] = out
    return json.dumps(j).encode(), n_split


def _build(reps=1, trivial=False, w3_fp8=None, conv_reps=1, conv1_only=False, conv2_seq=False, no_collective=False, hw_loop=False, empty_reps=0):
    YC = YC_HOST
    NQ = H1 // YC
    import concourse.bass as bass
    import concourse.mybir as mybir
    import concourse.tile as tile
    from concourse.masks import make_identity

    F32, F16, BF16 = mybir.dt.float32, mybir.dt.float16, mybir.dt.bfloat16
    AF = mybir.ActivationFunctionType
    ALU = mybir.AluOpType
    AX = mybir.AxisListType



    nc = bass.Bass("TRN2", target_bir_lowering=False, num_devices=NCORES)

    xb3 = nc.dram_tensor(
        "xb3", (H1 // YC_HOST, 2, 9, 4, YC_HOST, W + 2), BF16,
        kind="ExternalInput").ap()
    w1t = nc.dram_tensor("w1t", (9, 3, 32), BF16, kind="ExternalInput").ap()
    b1 = nc.dram_tensor("b1", (32, 1), F32, kind="ExternalInput").ap()
    w2t = nc.dram_tensor("w2t", (32, 3, 3, 64), BF16, kind="ExternalInput").ap()
    b2 = nc.dram_tensor("b2", (64, 1), F32, kind="ExternalInput").ap()
    wmt = nc.dram_tensor("wmt", (64, OPC), F32, kind="ExternalInput").ap()
    F8 = mybir.dt.float8e4
    w3s = nc.dram_tensor("w3s", (128, TOTC), F8, kind="ExternalInput").ap()
    wct = nc.dram_tensor("wct", (128, NWCH, OPC), F16, kind="ExternalInput").ap()
    pjt = nc.dram_tensor("pjt", (64, NPAIR_PAD), F16, kind="ExternalInput").ap()
    pkt = nc.dram_tensor("pkt", (64, NPAIR_PAD), F16, kind="ExternalInput").ap()
    bias3 = nc.dram_tensor("bias3", (3, OPC), F32, kind="ExternalInput").ap()
    out = nc.dram_tensor("out", (B, OPC), F32, kind="ExternalOutput").ap()
    feat_loc = nc.dram_tensor("feat_loc", (BL, FEAT), F32).ap()
    feat_all = nc.dram_tensor("feat_all", (B, FEAT), F32, addr_space="Shared").ap()

    if trivial:
        with tile.TileContext(nc) as tc:
            with tc.tile_pool(name="tp", bufs=1) as tp:
                z = tp.tile([B, OPC], F32)
                nc.vector.memset(z[:], 0.0)
                nc.sync.dma_start(out[:], z[:])
        nc.finalize()
        fixed, _ = _split_multiwait_json(nc.to_json_bytes())
        nc.to_json_bytes = lambda: fixed
        return nc

    with tile.TileContext(nc) as tc:
        with (
            tc.tile_pool(name="w3pool", bufs=1) as w3pool,
            tc.tile_pool(name="consts", bufs=1) as consts,
            tc.tile_pool(name="fcsingle", bufs=1) as fcsingle,
            tc.tile_pool(name="fcwork", bufs=4) as fcwork,
        ):
            # ---- constants ----
            w1sb = consts.tile([64, 3, 32], BF16)       # [(32r)+(dy,ci), dx, o]
            for r in range(2):
                nc.sync.dma_start(w1sb[32 * r:32 * r + 9, :, :], w1t[:])
            w2sb = consts.tile([128, 3, 3, 64], BF16)   # [(32c)+ci, dy, dx, o]
            for c in range(4):
                nc.sync.dma_start(w2sb[32 * c:32 * c + 32], w2t[:])
            bias1 = consts.tile([128, 1], F32)          # b1[cout] at 32c+cout
            nc.sync.dma_start(
                bias1[:],
                bass.AP(tensor=b1.tensor, offset=0, ap=[[0, 4], [1, 32], [1, 1]]),
            )
            bias2 = consts.tile([64, 1], F32)
            nc.sync.dma_start(bias2[:], b2[:])
            wmsb = consts.tile([64, OPC], F32)
            nc.sync.dma_start(wmsb[:], wmt[:])
            wcsb = consts.tile([128, NWCH, OPC], F16)   # [p, kt, o] half-sym wc
            nc.sync.dma_start(wcsb[:], wct[:])
            pjsb = consts.tile([64, NPAIR_PAD], F16)    # gather matrices
            nc.sync.dma_start(pjsb[:], pjt[:])
            pksb = consts.tile([64, NPAIR_PAD], F16)
            nc.sync.dma_start(pksb[:], pkt[:])
            bias3sb = consts.tile([64, 3, OPC], F32)
            nc.sync.dma_start(
                bias3sb[:],
                bass.AP(tensor=bias3.tensor, offset=0,
                        ap=[[0, 64], [OPC, 3], [1, OPC]]),
            )
            ident = consts.tile([64, 64], F32)
            make_identity(nc, ident[:])
            zeros = consts.tile([128, 448], BF16)
            nc.vector.memset(zeros[:], 0.0)
            bsum = consts.tile([64, OPC], F32)
            nc.vector.tensor_reduce(bsum[:], bias3sb[:].transpose([0, 2, 1]),
                                    AX.X, op=ALU.add)
            featparts = consts.tile([64, BL, 7], F32)
            if conv1_only:
                nc.vector.memset(featparts[:], 0.0)

            # =============== conv trunk (8 local images) ===============
            # conv1: K=(dy,ci)=9 at row groups {0,32}; 4 images per col group.
            # image assignment: img = 2*c + r  (c: col group, r: row group/bank)
            with (
                tc.tile_pool(name="conv", bufs=2) as conv,
                tc.tile_pool(name="h1p", bufs=1) as h1p,
            ):
                h1 = h1p.tile([128, 2, H1 + 2, W1 + 2], BF16)  # [(32c)+co, r, y+1, x+1]
                nc.vector.memset(h1[:, :, 0:1, :], 0.0)        # top pad row
                nc.vector.memset(h1[:, :, :, 0:1], 0.0)        # left pad col

                import contextlib

                if hw_loop and conv_reps > 1:
                    conv_loop_cm = tc.For_i(0, conv_reps, 1, name="convloop")
                    conv_iters = 1
                else:
                    conv_loop_cm = contextlib.nullcontext()
                    conv_iters = conv_reps
                with conv_loop_cm:
                 for _crep in range(conv_iters):
                    cpsum_cm = tc.tile_pool(name="cpsum", bufs=3, space="PSUM")
                    cpsum = cpsum_cm.__enter__()
                    for q in range(NQ):
                        a1 = conv.tile([64, 4, YC, W + 2], BF16, tag="a1")
                        for r in range(2):
                            nc.sync.dma_start(
                                a1[32 * r:32 * r + 9, :, :, :], xb3[q, r])
                        for s in range(YC // 4):
                            ps1 = cpsum.tile([128, 2, 512], F32, tag="cpsum")
                            for dx in range(3):
                                for r in range(2):
                                    for c in range(4):
                                        rhs = a1[32 * r:32 * r + 9, c,
                                                 4 * s:4 * s + 4, dx:dx + 2 * W1:2]
                                        nc.tensor.matmul(
                                            ps1[32 * c:32 * c + 32, r, 0:448],
                                            w1sb[32 * r:32 * r + 9, dx, :],
                                            rhs,
                                            start=(dx == 0), stop=(dx == 2),
                                            tile_position=(32 * r, 32 * c),
                                            skip_group_check=True,
                                        )
                            ybase = 1 + q * YC + 4 * s
                            for r in range(2):
                                src = ps1[:, r, 0:448].rearrange("p (y x) -> p y x", y=4)
                                dst = h1[:, r, ybase:ybase + 4, 1:113]
                                if r == 0:
                                    nc.scalar.activation(dst, src, AF.Relu,
                                                         bias=bias1[:], scale=1.0)
                                else:
                                    nc.vector.scalar_tensor_tensor(
                                        dst, src, bias1[:],
                                        zeros[:].rearrange("p (y x) -> p y x", y=4),
                                        op0=ALU.add, op1=ALU.max,
                                    )

                    cpsum_cm.__exit__(None, None, None)

                    if conv1_only:
                        continue
                    # conv2: K=ci=32 at row groups {0,32,64,96} (4 images = 4 col
                    # groups of h1), M=64, 9 taps accumulate; relu+GAP via ACT.
                    with tc.tile_pool(name="c2psum", bufs=2, space="PSUM") as c2psum:
                        trash = consts.tile([64, 448], BF16)
                        trash2 = consts.tile([64, 448], BF16)
                        for r in range(2):
                            for sc in range(7):
                                ps2 = c2psum.tile([64, 4, 512], F32, tag="c2psum")
                                taporder = (
                                    [(c, dy, dx) for c in range(4)
                                     for dy in range(3) for dx in range(3)]
                                    if conv2_seq else
                                    [(c, dy, dx) for dy in range(3)
                                     for dx in range(3) for c in range(4)])
                                for c, dy, dx in taporder:
                                    rhs = h1[32 * c:32 * c + 32, r,
                                             2 * (8 * sc) + dy:2 * (8 * sc) + dy + 16:2,
                                             dx:dx + 2 * W2:2]
                                    nc.tensor.matmul(
                                        ps2[0:64, c, 0:448],
                                        w2sb[32 * c:32 * c + 32, dy, dx, :],
                                        rhs,
                                        start=(dy == 0 and dx == 0),
                                        stop=(dy == 2 and dx == 2),
                                        tile_position=(32 * c, 0),
                                    )
                                for c in range(4):
                                    img = 2 * c + r
                                    if c % 2 == 0:
                                        nc.scalar.activation(
                                            trash[:], ps2[0:64, c, 0:448], AF.Relu,
                                            bias=bias2[:], scale=1.0,
                                            accum_out=featparts[:, img, sc:sc + 1],
                                        )
                                    else:
                                        nc.vector.scalar_tensor_tensor(
                                            trash2[:], ps2[0:64, c, 0:448], bias2[:],
                                            zeros[0:64, :],
                                            op0=ALU.add, op1=ALU.max,
                                            accum_out=featparts[:, img, sc:sc + 1],
                                        )

            # featT_loc[f, img] = GAP * sum_sc featparts
            featTl = fcsingle.tile([64, BL], F32, tag="featTl")
            nc.vector.tensor_reduce(featTl[:], featparts[:], AX.X, op=ALU.add)
            nc.vector.tensor_scalar_mul(featTl[:], featTl[:], GAP)
            nc.sync.dma_start(feat_loc[:].transpose([1, 0]), featTl[:])

            if no_collective:
                for c in range(NCORES):
                    nc.sync.dma_start(feat_all[BL * c:BL * (c + 1), :], feat_loc[:])
            else:
                nc.gpsimd.collective_compute(
                    "AllGather", ALU.bypass,
                    replica_groups=[list(range(NCORES))],
                    ins=[feat_loc[:]], outs=[feat_all[:]],
                )

            if empty_reps:
                scratch = fcsingle.tile([64, 16], F32, tag="scratch")
                with tc.For_i(0, empty_reps, 1, name="emptyloop"):
                    nc.vector.memset(scratch[:], 0.0)

            import contextlib
            if hw_loop and reps > 1:
                fc_loop_cm = tc.For_i(0, reps, 1, name="fcloop")
                fc_iters = 1
            else:
                fc_loop_cm = contextlib.nullcontext()
                fc_iters = reps
            with fc_loop_cm:
             for _rep in range(fc_iters):
                # =============== fc prep ===============
                feat = fcsingle.tile([64, 64], F32, tag="feat")
                nc.sync.dma_start(feat[:], feat_all[:])
                mean = fcsingle.tile([64, 1], F32, tag="mean")
                nc.vector.tensor_reduce(mean[:], feat[:], AX.X, op=ALU.add)
                nc.vector.tensor_scalar_mul(mean[:], mean[:], 1.0 / FEAT)
                fc = fcsingle.tile([64, 64], F32, tag="fc")
                nc.vector.tensor_scalar_sub(fc[:], feat[:], mean[:])
                # fcsel[p, c] = fc[p%64, c + 32*(p//64)] / 64 -- contraction
                # scales (1/64 undoes the 8x fc scaling baked into the covU
                # gathers to keep fp8 covU out of the subnormal range)
                fcdiv = fcsingle.tile([64, 64], F32, tag="fcdiv")
                nc.vector.tensor_scalar_mul(fcdiv[:], fc[:], 1.0 / 64.0)
                fcsel = fcsingle.tile([128, 32], F32, tag="fcsel")
                nc.sync.dma_start(fcsel[0:64, :], fcdiv[:, 0:32])
                nc.sync.dma_start(fcsel[64:128, :], fcdiv[:, 32:64])

                covUT8 = fcsingle.tile([128, NCH, 64], F8, tag="covUT8")
                covUT16 = fcsingle.tile([128, NCH, 64], F16, tag="covUT16")
                fcT16 = fcsingle.tile([64, 64], F16, tag="fcT16")
                featT = fcsingle.tile([64, 64], F32, tag="featT")
                pwcsb = fcsingle.tile([64, OPC], F32, tag="pwcsb")
                thirdparts = fcsingle.tile([128, 8, OPC], F32, tag="thparts")

                # prefetch the whole folded-w3 stream in a few medium DMAs
                # (DMA-issue on SP costs ~650ns each; 58 small DMAs would
                # serialize -- but >16KB/partition DMAs inside For_i wedge
                # the device, so chunk at 8 blocks per DMA)
                DMACH = 8
                w3big = w3pool.tile([128, NBLK * 1024], F8, tag="w3big")
                for t0 in range(0, NBLK, DMACH):
                    t1 = min(t0 + DMACH, NBLK)
                    nc.sync.dma_start(w3big[:, 1024 * t0:1024 * t1],
                                      w3s[:, 1024 * t0:1024 * t1])
                w3tiles = [w3big[:, 1024 * t:1024 * (t + 1)]
                           for t in range(NBLK)]

                # packed covU (pairs j<=k) via gather-matmuls, in transposed
                # K-major layout: covUT[p, kt, b] = fc[b, pj]*fc[b, pk]
                with tc.tile_pool(name="prepT", bufs=2, space="PSUM") as prepT, \
                     tc.tile_pool(name="prepA", bufs=2, space="PSUM") as prepA, \
                     tc.tile_pool(name="prepB", bufs=2, space="PSUM") as prepB:
                    pT = prepT.tile([128, 64], F32, tag="pT")
                    nc.tensor.transpose(pT[0:64, :], fc[:], ident[:])
                    nc.vector.tensor_scalar_mul(fcT16[:], pT[0:64, :], 8.0)
                    pT2 = prepT.tile([128, 64], F32, tag="pT")
                    nc.tensor.transpose(pT2[0:64, :], feat[:], ident[:])
                    nc.vector.tensor_copy(featT[:], pT2[0:64, :])
                    for q in range(5):
                        nch = min(4, NCH - 4 * q)
                        pA = prepA.tile([128, 4, 64], F32, tag="pA")
                        pB = prepB.tile([128, 4, 64], F32, tag="pB")
                        for c in range(nch):
                            ch = 4 * q + c
                            nc.tensor.matmul(pA[:, c, :],
                                             pjsb[:, 128 * ch:128 * (ch + 1)],
                                             fcT16[:], start=True, stop=True)
                            nc.tensor.matmul(pB[:, c, :],
                                             pksb[:, 128 * ch:128 * (ch + 1)],
                                             fcT16[:], start=True, stop=True)
                        # (two PSUM operands in one DVE op is rejected; copy
                        # the A side to SBUF via the scalar engine first)
                        sA = fcwork.tile([128, 4, 64], F16, tag="sA")
                        nc.scalar.activation(
                            sA[:, 0:nch, :].rearrange("p a b -> p (a b)"),
                            pA[:, 0:nch, :].rearrange("p a b -> p (a b)"),
                            AF.Copy)
                        nc.vector.tensor_mul(covUT16[:, 4 * q:4 * q + nch, :],
                                             sA[:, 0:nch, :], pB[:, 0:nch, :])
                        nc.vector.tensor_mul(covUT8[:, 4 * q:4 * q + nch, :],
                                             sA[:, 0:nch, :], pB[:, 0:nch, :])

                # wc (half-sym) + wm pre-pass
                with tc.tile_pool(name="wcp", bufs=1, space="PSUM") as wcp:
                    pwc = wcp.tile([64, OPC], F32)
                    for kt in range(NWCH):
                        nc.tensor.matmul(pwc[:], covUT16[:, kt, :], wcsb[:, kt, :],
                                         start=(kt == 0), stop=False)
                    nc.tensor.matmul(pwc[:], featT[:], wmsb[:], start=False,
                                     stop=True)
                    nc.vector.tensor_copy(pwcsb[:], pwc[:])

                # ---- DoubleRow block-sparse w3 stream ----
                with tc.tile_pool(name="drpsum", bufs=1, space="PSUM") as drpsum:
                    pb = []
                    for t in range(8):
                        pbt = drpsum.tile([128, 512], F32, tag=f"pb{t}",
                                          name=f"pb{t}")
                        pb.append(pbt)
                    idx = 0
                    for g in range(NG):
                        lhs = covUT8[:, 2 * g:2 * g + 2, :]
                        for b in range(NBG[g]):
                            t, hl = b % 8, b // 8
                            rhs2 = w3tiles[idx].rearrange(
                                "p (a n) -> p a n", a=2)
                            if hl == 0:
                                nc.tensor.matmul(
                                    pb[t][0:64, :], lhs, rhs2,
                                    start=(g == 0), stop=(g == GMAX[b]),
                                    perf_mode=mybir.MatmulPerfMode.DoubleRow,
                                    tile_position=(0, 0),
                                    skip_group_check=True,
                                )
                            else:
                                # DoubleRow + column tile_position 64 fails
                                # walrus ISA check; these 12 shallow blocks
                                # run as plain fp8 matmuls instead
                                for kt in range(2):
                                    nc.tensor.matmul(
                                        pb[t][64:128, :],
                                        covUT8[:, 2 * g + kt, :],
                                        rhs2[:, kt, :],
                                        start=(g == 0 and kt == 0),
                                        stop=(g == GMAX[b] and kt == 1),
                                        tile_position=(0, 64),
                                        skip_group_check=True,
                                    )
                            idx += 1

                    # contraction over i: third[b,o] += fc[b,i] * T1[b,(i,o)]
                    # tiles in psum-readiness order; 2 tiles routed via Pool
                    order = sorted(range(8), key=lambda t: GMAX[t])
                    pool_tiles = set(order[1:3])
                    for t in order:
                        if t in pool_tiles:
                            tmpf = fcwork.tile([128, 4, OPC], F16, tag="tmpf")
                            nc.scalar.activation(
                                tmpf[:].rearrange("p a o -> p (a o)"),
                                pb[t][:], AF.Copy)
                            tmm = fcwork.tile([128, 4, OPC], F16, tag="tmm")
                            nc.gpsimd.tensor_mul(
                                tmm[:], tmpf[:],
                                fcsel[:, 4 * t:4 * t + 4].unsqueeze(2)
                                .broadcast_to([128, 4, OPC]))
                            nc.vector.tensor_reduce(
                                thirdparts[:, t, :],
                                tmm[:].transpose([0, 2, 1]),
                                AX.X, op=ALU.add)
                        else:
                            tmpf = fcwork.tile([128, 4, OPC], F16, tag="tmpf")
                            for il in range(4):
                                nc.scalar.activation(
                                    tmpf[:, il, :],
                                    pb[t][:, il * OPC:(il + 1) * OPC],
                                    AF.Copy,
                                    scale=fcsel[:, 4 * t + il:4 * t + il + 1])
                            nc.vector.tensor_reduce(
                                thirdparts[:, t, :],
                                tmpf[:].transpose([0, 2, 1]),
                                AX.X, op=ALU.add)

                # ---- final assembly ----
                thsum = fcsingle.tile([128, OPC], F32, tag="thsum")
                nc.vector.tensor_reduce(
                    thsum[:], thirdparts[:].transpose([0, 2, 1]),
                    AX.X, op=ALU.add)
                thup = fcsingle.tile([64, OPC], F32, tag="thup")
                nc.sync.dma_start(thup[:], thsum[64:128, :])
                acc = fcsingle.tile([64, OPC], F32, tag="acc")
                nc.vector.tensor_add(acc[:], thsum[0:64, :], thup[:])
                nc.vector.tensor_add(acc[:], acc[:], pwcsb[:])
                nc.vector.tensor_add(acc[:], acc[:], bsum[:])
                nc.sync.dma_start(out[:], acc[:])

    nc.finalize()
    fixed, n_split = _split_multiwait_json(nc.to_json_bytes())
    nc.to_json_bytes = lambda: fixed
    return nc


_NC_CACHE = None


def _get_nc():
    global _NC_CACHE
    if _NC_CACHE is None:
        _NC_CACHE = _build()
    return _NC_CACHE


def _prepare_in_maps(inputs):
    x = np.asarray(inputs["x"])
    w1 = np.asarray(inputs["w1"])
    b1 = np.asarray(inputs["b1"])
    w2 = np.asarray(inputs["w2"])
    b2 = np.asarray(inputs["b2"])
    wm = np.asarray(inputs["wm"])
    bm = np.asarray(inputs["bm"])
    wc = np.asarray(inputs["wc"])
    bc = np.asarray(inputs["bc"])
    w3 = np.asarray(inputs["w3"])
    b3 = np.asarray(inputs["b3"])

    bf16 = ml_dtypes.bfloat16
    xb = x.astype(bf16)
    # pre-tiled conv1 input: xb3[core][q, r, (dy,ci), c, y, 1+w]
    #   = x[core*8 + 2c + r, ci, 2*(16q+y)+dy-1, w]  (zeros at pads)
    NQH = H1 // YC_HOST
    xb3 = np.zeros((NCORES, NQH, 2, 9, 4, YC_HOST, W + 2), dtype=bf16)
    rows = 2 * (np.arange(NQH * YC_HOST).reshape(NQH, YC_HOST))[None, None, :, :] \
        + np.arange(3).reshape(1, 3, 1, 1) - 1          # [1, dy, q, y]
    valid = (rows >= 0) & (rows < H)
    rowsc = np.clip(rows, 0, H - 1)
    # gather: xb3[n, q, r, dy*3+ci, c, y, 1:] = xb[n*8+2c+r, ci, rows[dy,q,y], :]
    for r in range(2):
        for c in range(4):
            img = xb[2 * c + r::BL, :, :, :]            # [NCORES, 3, H, W]
            g = img[:, :, rowsc[0], :]                  # [NCORES, 3, dy, q, y, W]
            g = g * valid[0][None, None, :, :, :, None].astype(bf16)
            xb3[:, :, r, :, c, :, 1:1 + W] = (
                g.transpose(0, 3, 2, 1, 4, 5).reshape(NCORES, NQH, 9, YC_HOST, W))
    w1t = np.ascontiguousarray(w1.transpose(2, 1, 3, 0)).reshape(9, 3, 32).astype(bf16)
    w2t = np.ascontiguousarray(w2.transpose(1, 2, 3, 0)).astype(bf16)
    b1r = np.ascontiguousarray(b1.reshape(32, 1)).astype(np.float32)
    b2r = np.ascontiguousarray(b2.reshape(64, 1)).astype(np.float32)

    # ---- symmetric fold of w3 ------------------------------------------
    fp8 = ml_dtypes.float8_e4m3
    OUT = NCORES * OPC
    w3r = w3.reshape(OUT, FEAT, FEAT, FEAT).astype(np.float32)
    s = w3r.copy()
    for perm in [(0, 1, 3, 2), (0, 2, 1, 3), (0, 2, 3, 1), (0, 3, 1, 2),
                 (0, 3, 2, 1)]:
        s += w3r.transpose(perm)
    ii, jj, kk = np.meshgrid(np.arange(FEAT), np.arange(FEAT),
                             np.arange(FEAT), indexing="ij")
    multc = 1.0 + (ii == jj) + (jj == kk) + 3 * ((ii == jj) & (jj == kk))
    s *= (1.0 / multc)[None]

    # static gather indices for the packed DoubleRow stream [128, TOTC]
    pair_arr = np.asarray(_PAIRS_P, np.int32)          # [2304, 2]
    real = np.arange(NPAIR_PAD) < NPAIR
    IDX = np.zeros((128, TOTC), np.int32)    # (i*64+j)*64+k monomial index
    OIDX = np.zeros((128, TOTC), np.int32)   # local output index o
    VALID = np.zeros((128, TOTC), bool)
    col = 0
    o_ = np.arange(OPC, dtype=np.int32)[None, None, None, :]
    for g in range(NG):
        seg = slice(g * 256, (g + 1) * 256)
        pjg = pair_arr[seg, 0].reshape(2, 128).T[:, :, None, None]  # [128,2,1,1]
        pkg = pair_arr[seg, 1].reshape(2, 128).T[:, :, None, None]
        rl = real[seg].reshape(2, 128).T[:, :, None, None]
        for b in range(NBG[g]):
            i_ = (4 * b + np.arange(4, dtype=np.int32))[None, None, :, None]
            v = (i_ <= pjg) & rl
            idx = (i_ * FEAT + pjg) * FEAT + pkg                    # [128,2,4,1]
            IDX[:, col:col + 1024] = np.broadcast_to(
                idx, (128, 2, 4, OPC)).reshape(128, 1024)
            OIDX[:, col:col + 1024] = np.broadcast_to(
                o_, (128, 2, 4, OPC)).reshape(128, 1024)
            VALID[:, col:col + 1024] = np.broadcast_to(
                v, (128, 2, 4, OPC)).reshape(128, 1024)
            col += 1024

    # wc half-sym fold: wchalf[o, j, k] (j<=k) = wc[o,j,k] + wc[o,k,j] offdiag
    wc_r = wc.reshape(OUT, FEAT, FEAT).astype(np.float32)
    wchalf = wc_r + wc_r.transpose(0, 2, 1)
    wchalf[:, np.arange(FEAT), np.arange(FEAT)] = \
        wc_r[:, np.arange(FEAT), np.arange(FEAT)]
    # wct layout [128 p, NWCH kt, OPC o]: pair m = kt*128+p
    wcj = pair_arr[:NWCH * 128, 0].reshape(NWCH, 128).T    # [128, kt]
    wck = pair_arr[:NWCH * 128, 1].reshape(NWCH, 128).T
    wcreal = real[:NWCH * 128].reshape(NWCH, 128).T        # [128, kt]

    # gather matrices (f16): pj/pk one-hot, zeroed on padding
    pjm = np.zeros((64, NPAIR_PAD), np.float16)
    pkm = np.zeros((64, NPAIR_PAD), np.float16)
    m_ = np.arange(NPAIR)
    pjm[pair_arr[:NPAIR, 0], m_] = 1.0
    pkm[pair_arr[:NPAIR, 1], m_] = 1.0

    in_maps = []
    for c in range(NCORES):
        osl = slice(OPC * c, OPC * (c + 1))
        base = (OPC * c + OIDX).astype(np.int64) * (FEAT ** 3) + IDX
        vals = s.reshape(-1)[base]
        vals[~VALID] = 0.0
        w3sc = vals.astype(fp8)
        wcc = wchalf[osl]                                   # [OPC, F, F]
        wctc = wcc[:, wcj, wck].transpose(1, 2, 0)          # [128, kt, OPC]
        # 1/64 undoes the 8x-scaled covU gathers (see kernel fcT16)
        wctc = (wctc * wcreal[:, :, None] / 64.0).astype(np.float16)
        in_maps.append({
            "xb3": np.ascontiguousarray(xb3[c]),
            "w1t": w1t,
            "b1": b1r,
            "w2t": w2t,
            "b2": b2r,
            "wmt": np.ascontiguousarray(wm[osl].T).astype(np.float32),
            "wct": np.ascontiguousarray(wctc),
            "w3s": np.ascontiguousarray(w3sc),
            "pjt": pjm,
            "pkt": pkm,
            "bias3": np.stack([bm[osl], bc[osl], b3[osl]]).astype(np.float32),
        })

    return in_maps


def kernel(**inputs):
    in_maps = _prepare_in_maps(inputs)
    from concourse.bass_utils import run_bass_kernel_spmd

    res = run_bass_kernel_spmd(_get_nc(), in_maps, core_ids=list(range(NCORES)))
    return np.concatenate([res.results[c]["out"] for c in range(NCORES)], axis=1)


if __name__ == "__main__":
    nc = _build()
    print("built OK; instructions:",
          sum(len(bb.instructions) for f in nc.m.functions for bb in f.blocks))
    if "compile" in sys.argv:
        import tempfile
        from concourse.bass_utils import compile_bass_kernel
        d = tempfile.mkdtemp()
        print("compiling in", d)
        print("NEFF:", compile_bass_kernel(nc, d))



# revision 39
# speedup vs baseline: 1.2585x; 1.2585x over previous
"""Trainium2 Bass kernel for nn_NoiseProjector.

Strategy (8 NeuronCores):
- Data-parallel conv trunk: each core runs conv1+conv2+GAP on 8 of 64 images.
- Tiny AllGather of the pooled features (8x64 -> 64x64 per core).
- Tensor-parallel FC stage: fc_third (w3) / fc_cov (wc) / fc_mean (wm) weights
  column-sharded over output_dim (128 outputs per core).  The third-order term
  is computed as  T1 = cov @ W3r.T  (W3r = w3 reshaped (OUT*64, 4096)) followed
  by a small contraction over i:  third[b,o] = sum_i fc[b,i] * T1[b, o*64+i].
- w3 is pre-transposed host-side to K-major layout and cast to fp8e4m3 (wc to
  fp16) so the dominant 1 GiB weight stream is clean contiguous DMA at 1/4 the
  bytes; measured end-to-end relative error ~1.3e-3.
- The conv1 input is pre-tiled on the host into the exact (dy,ci)-replicated
  SBUF layout (one contiguous DMA per y-chunk instead of 336 small strided
  DMAs, which cost ~0.7 us each).
- Host concatenates the 8 per-core (64,128) outputs into (64,1024).
"""

import sys

sys.path.insert(0, "/opt/trn_rl_repo")

import numpy as np
import ml_dtypes

B = 64          # global batch
BL = 8          # images per core
NCORES = 8
OPC = 128       # outputs per core
FEAT = 64
JK = 4096       # FEAT*FEAT contraction
H, W = 224, 224
H1, W1 = 112, 112   # conv1 out
H2, W2 = 56, 56     # conv2 out
YC_HOST = 8         # conv1 y-rows per chunk (host pre-tiled layout)
GAP = 1.0 / (H2 * W2)

# --- symmetric-fold third-order stage tables -------------------------------
# pairs (j,k) j<=k sorted descending by j; K-chunks of 128, DoubleRow pairs
# of chunks (256 pairs per "g" group), 9 groups (2304 slots, 2080 real).
NG = 9                      # DoubleRow K-groups
NCH = 2 * NG                # 18 K-chunks of 128
NPAIR_PAD = NCH * 128       # 2304
_PAIRS = [(j, k) for j in range(63, -1, -1) for k in range(j, 64)]
NPAIR = len(_PAIRS)         # 2080
_PAIRS_P = _PAIRS + [(0, 0)] * (NPAIR_PAD - NPAIR)
# blocks of 4 i-values x 128 outputs (512 cols) active per group
NBG = []
for _g in range(NG):
    _jmax = max(j for j, _ in _PAIRS[_g * 256:min((_g + 1) * 256, NPAIR)])
    NBG.append(_jmax // 4 + 1)
NBLK = sum(NBG)             # 58 stream tiles
TOTC = NBLK * 1024          # stream columns (fp8 bytes per partition row)
# gmax[b] = last group contributing to i-block b
GMAX = [max(g for g in range(NG) if NBG[g] >= b + 1) for b in range(16)]
NWCH = 17                   # chunks carrying real pairs (2080 <= 17*128)
NBLK_A = 29                 # stream split: blocks in the first big DMA



def _split_multiwait_json(raw):
    """This walrus build accepts only ONE sync wait per instruction.  Split any
    multi-wait instruction into single-wait EventSemaphore ops ahead of it (the
    engine is in-order, so chained waits are equivalent)."""
    import json

    j = json.loads(raw)
    n_split = 0
    for f in j["functions"]:
        for bb in f["blocks"]:
            insts = bb.get("instructions")
            if not insts:
                continue
            out = []
            changed = False
            for ins in insts:
                si = ins.get("sync_info")
                waits = si.get("on_wait") if si else None
                if waits and len(waits) > 1:
                    changed = True
                    keep = None
                    for w in waits:
                        if w.get("wait_reg") is not None:
                            keep = w
                    if keep is None:
                        keep = waits[-1]
                    rest = [w for w in waits if w is not keep]
                    for k, w in enumerate(rest):
                        n_split += 1
                        out.append({
                            "engine": ins["engine"], "ins": [], "outs": [],
                            "name": f"{ins['name']}-sw{k}",
                            "opcode": "EventSemaphore",
                            "sync_info": {"on_update": [], "on_wait": [w]},
                        })
                    si["on_wait"] = [keep]
                out.append(ins)
            if changed:
                bb["instructions"] = out
    return json.dumps(j).encode(), n_split


def _build(reps=1, trivial=False, w3_fp8=None, conv_reps=1, conv1_only=False, conv2_seq=False, no_collective=False, hw_loop=False, empty_reps=0):
    YC = YC_HOST
    NQ = H1 // YC
    import concourse.bass as bass
    import concourse.mybir as mybir
    import concourse.tile as tile
    from concourse.masks import make_identity

    F32, F16, BF16 = mybir.dt.float32, mybir.dt.float16, mybir.dt.bfloat16
    AF = mybir.ActivationFunctionType
    ALU = mybir.AluOpType
    AX = mybir.AxisListType



    nc = bass.Bass("TRN2", target_bir_lowering=False, num_devices=NCORES)

    xb3 = nc.dram_tensor(
        "xb3", (H1 // YC_HOST, 2, 9, 4, YC_HOST, W + 2), BF16,
        kind="ExternalInput").ap()
    w1t = nc.dram_tensor("w1t", (9, 3, 32), BF16, kind="ExternalInput").ap()
    b1 = nc.dram_tensor("b1", (32, 1), F32, kind="ExternalInput").ap()
    w2t = nc.dram_tensor("w2t", (32, 3, 3, 64), BF16, kind="ExternalInput").ap()
    b2 = nc.dram_tensor("b2", (64, 1), F32, kind="ExternalInput").ap()
    wmt = nc.dram_tensor("wmt", (64, OPC), F32, kind="ExternalInput").ap()
    F8 = mybir.dt.float8e4
    w3s = nc.dram_tensor("w3s", (128, TOTC), F8, kind="ExternalInput").ap()
    wct = nc.dram_tensor("wct", (128, NWCH, OPC), F16, kind="ExternalInput").ap()
    pjt = nc.dram_tensor("pjt", (64, NPAIR_PAD), F16, kind="ExternalInput").ap()
    pkt = nc.dram_tensor("pkt", (64, NPAIR_PAD), F16, kind="ExternalInput").ap()
    bias3 = nc.dram_tensor("bias3", (3, OPC), F32, kind="ExternalInput").ap()
    out = nc.dram_tensor("out", (B, OPC), F32, kind="ExternalOutput").ap()
    feat_loc = nc.dram_tensor("feat_loc", (BL, FEAT), F32).ap()
    feat_all = nc.dram_tensor("feat_all", (B, FEAT), F32, addr_space="Shared").ap()

    if trivial:
        with tile.TileContext(nc) as tc:
            with tc.tile_pool(name="tp", bufs=1) as tp:
                z = tp.tile([B, OPC], F32)
                nc.vector.memset(z[:], 0.0)
                nc.sync.dma_start(out[:], z[:])
        nc.finalize()
        fixed, _ = _split_multiwait_json(nc.to_json_bytes())
        nc.to_json_bytes = lambda: fixed
        return nc

    with tile.TileContext(nc) as tc:
        with (
            tc.tile_pool(name="w3pool", bufs=1) as w3pool,
            tc.tile_pool(name="consts", bufs=1) as consts,
            tc.tile_pool(name="fcsingle", bufs=1) as fcsingle,
            tc.tile_pool(name="fcwork", bufs=4) as fcwork,
        ):
            # ---- constants ----
            w1sb = consts.tile([64, 3, 32], BF16)       # [(32r)+(dy,ci), dx, o]
            for r in range(2):
                nc.sync.dma_start(w1sb[32 * r:32 * r + 9, :, :], w1t[:])
            w2sb = consts.tile([128, 3, 3, 64], BF16)   # [(32c)+ci, dy, dx, o]
            for c in range(4):
                nc.sync.dma_start(w2sb[32 * c:32 * c + 32], w2t[:])
            bias1 = consts.tile([128, 1], F32)          # b1[cout] at 32c+cout
            nc.sync.dma_start(
                bias1[:],
                bass.AP(tensor=b1.tensor, offset=0, ap=[[0, 4], [1, 32], [1, 1]]),
            )
            bias2 = consts.tile([64, 1], F32)
            nc.sync.dma_start(bias2[:], b2[:])
            wmsb = consts.tile([64, OPC], F32)
            nc.sync.dma_start(wmsb[:], wmt[:])
            wcsb = consts.tile([128, NWCH, OPC], F16)   # [p, kt, o] half-sym wc
            nc.sync.dma_start(wcsb[:], wct[:])
            pjsb = consts.tile([64, NPAIR_PAD], F16)    # gather matrices
            nc.sync.dma_start(pjsb[:], pjt[:])
            pksb = consts.tile([64, NPAIR_PAD], F16)
            nc.sync.dma_start(pksb[:], pkt[:])
            bias3sb = consts.tile([64, 3, OPC], F32)
            nc.sync.dma_start(
                bias3sb[:],
                bass.AP(tensor=bias3.tensor, offset=0,
                        ap=[[0, 64], [OPC, 3], [1, OPC]]),
            )
            ident = consts.tile([64, 64], F32)
            make_identity(nc, ident[:])
            zeros = consts.tile([128, 448], BF16)
            nc.vector.memset(zeros[:], 0.0)
            bsum = consts.tile([64, OPC], F32)
            nc.vector.tensor_reduce(bsum[:], bias3sb[:].transpose([0, 2, 1]),
                                    AX.X, op=ALU.add)
            featparts = consts.tile([64, BL, 7], F32)
            if conv1_only:
                nc.vector.memset(featparts[:], 0.0)

            # =============== conv trunk (8 local images) ===============
            # conv1: K=(dy,ci)=9 at row groups {0,32}; 4 images per col group.
            # image assignment: img = 2*c + r  (c: col group, r: row group/bank)
            with (
                tc.tile_pool(name="conv", bufs=2) as conv,
                tc.tile_pool(name="h1p", bufs=1) as h1p,
            ):
                h1 = h1p.tile([128, 2, H1 + 2, W1 + 2], BF16)  # [(32c)+co, r, y+1, x+1]
                nc.vector.memset(h1[:, :, 0:1, :], 0.0)        # top pad row
                nc.vector.memset(h1[:, :, :, 0:1], 0.0)        # left pad col

                import contextlib

                UNROLL = 8
                if hw_loop and conv_reps > 1:
                    assert (conv_reps - 1) % UNROLL == 0
                    conv_loop_cm = tc.For_i(0, (conv_reps - 1) // UNROLL, 1,
                                            name="convloop")
                    conv_iters = UNROLL
                else:
                    conv_loop_cm = contextlib.nullcontext()
                    conv_iters = conv_reps
                with conv_loop_cm:
                 for _crep in range(conv_iters):
                    cpsum_cm = tc.tile_pool(name="cpsum", bufs=3, space="PSUM")
                    cpsum = cpsum_cm.__enter__()
                    for q in range(NQ):
                        a1 = conv.tile([64, 4, YC, W + 2], BF16, tag="a1")
                        for r in range(2):
                            nc.sync.dma_start(
                                a1[32 * r:32 * r + 9, :, :, :], xb3[q, r])
                        for s in range(YC // 4):
                            ps1 = cpsum.tile([128, 2, 512], F32, tag="cpsum")
                            for dx in range(3):
                                for r in range(2):
                                    for c in range(4):
                                        rhs = a1[32 * r:32 * r + 9, c,
                                                 4 * s:4 * s + 4, dx:dx + 2 * W1:2]
                                        nc.tensor.matmul(
                                            ps1[32 * c:32 * c + 32, r, 0:448],
                                            w1sb[32 * r:32 * r + 9, dx, :],
                                            rhs,
                                            start=(dx == 0), stop=(dx == 2),
                                            tile_position=(32 * r, 32 * c),
                                            skip_group_check=True,
                                        )
                            ybase = 1 + q * YC + 4 * s
                            for r in range(2):
                                src = ps1[:, r, 0:448].rearrange("p (y x) -> p y x", y=4)
                                dst = h1[:, r, ybase:ybase + 4, 1:113]
                                if r == 0:
                                    nc.scalar.activation(dst, src, AF.Relu,
                                                         bias=bias1[:], scale=1.0)
                                else:
                                    nc.vector.scalar_tensor_tensor(
                                        dst, src, bias1[:],
                                        zeros[:].rearrange("p (y x) -> p y x", y=4),
                                        op0=ALU.add, op1=ALU.max,
                                    )

                    cpsum_cm.__exit__(None, None, None)

                    if conv1_only:
                        continue
                    # conv2: K=ci=32 at row groups {0,32,64,96} (4 images = 4 col
                    # groups of h1), M=64, 9 taps accumulate; relu+GAP via ACT.
                    with tc.tile_pool(name="c2psum", bufs=2, space="PSUM") as c2psum:
                        trash = consts.tile([64, 448], BF16)
                        trash2 = consts.tile([64, 448], BF16)
                        for r in range(2):
                            for sc in range(7):
                                ps2 = c2psum.tile([64, 4, 512], F32, tag="c2psum")
                                taporder = (
                                    [(c, dy, dx) for c in range(4)
                                     for dy in range(3) for dx in range(3)]
                                    if conv2_seq else
                                    [(c, dy, dx) for dy in range(3)
                                     for dx in range(3) for c in range(4)])
                                for c, dy, dx in taporder:
                                    rhs = h1[32 * c:32 * c + 32, r,
                                             2 * (8 * sc) + dy:2 * (8 * sc) + dy + 16:2,
                                             dx:dx + 2 * W2:2]
                                    nc.tensor.matmul(
                                        ps2[0:64, c, 0:448],
                                        w2sb[32 * c:32 * c + 32, dy, dx, :],
                                        rhs,
                                        start=(dy == 0 and dx == 0),
                                        stop=(dy == 2 and dx == 2),
                                        tile_position=(32 * c, 0),
                                    )
                                for c in range(4):
                                    img = 2 * c + r
                                    if c % 2 == 0:
                                        nc.scalar.activation(
                                            trash[:], ps2[0:64, c, 0:448], AF.Relu,
                                            bias=bias2[:], scale=1.0,
                                            accum_out=featparts[:, img, sc:sc + 1],
                                        )
                                    else:
                                        nc.vector.scalar_tensor_tensor(
                                            trash2[:], ps2[0:64, c, 0:448], bias2[:],
                                            zeros[0:64, :],
                                            op0=ALU.add, op1=ALU.max,
                                            accum_out=featparts[:, img, sc:sc + 1],
                                        )

            # featT_loc[f, img] = GAP * sum_sc featparts
            featTl = fcsingle.tile([64, BL], F32, tag="featTl")
            nc.vector.tensor_reduce(featTl[:], featparts[:], AX.X, op=ALU.add)
            nc.vector.tensor_scalar_mul(featTl[:], featTl[:], GAP)
            nc.sync.dma_start(feat_loc[:].transpose([1, 0]), featTl[:])

            if no_collective:
                for c in range(NCORES):
                    nc.sync.dma_start(feat_all[BL * c:BL * (c + 1), :], feat_loc[:])
            else:
                nc.gpsimd.collective_compute(
                    "AllGather", ALU.bypass,
                    replica_groups=[list(range(NCORES))],
                    ins=[feat_loc[:]], outs=[feat_all[:]],
                )

            if empty_reps:
                scratch = fcsingle.tile([64, 16], F32, tag="scratch")
                with tc.For_i(0, empty_reps, 1, name="emptyloop"):
                    nc.vector.memset(scratch[:], 0.0)

            import contextlib
            if hw_loop and reps > 1:
                assert (reps - 1) % 8 == 0
                fc_loop_cm = tc.For_i(0, (reps - 1) // 8, 1, name="fcloop")
                fc_iters = 8
            else:
                fc_loop_cm = contextlib.nullcontext()
                fc_iters = reps
            with fc_loop_cm:
             for _rep in range(fc_iters):
                # =============== fc prep ===============
                feat = fcsingle.tile([64, 64], F32, tag="feat")
                nc.sync.dma_start(feat[:], feat_all[:])
                mean = fcsingle.tile([64, 1], F32, tag="mean")
                nc.vector.tensor_reduce(mean[:], feat[:], AX.X, op=ALU.add)
                nc.vector.tensor_scalar_mul(mean[:], mean[:], 1.0 / FEAT)
                fc = fcsingle.tile([64, 64], F32, tag="fc")
                nc.vector.tensor_scalar_sub(fc[:], feat[:], mean[:])
                # fcsel[p, c] = fc[p%64, c + 32*(p//64)] / 64 -- contraction
                # scales (1/64 undoes the 8x fc scaling baked into the covU
                # gathers to keep fp8 covU out of the subnormal range)
                fcdiv = fcsingle.tile([64, 64], F32, tag="fcdiv")
                nc.vector.tensor_scalar_mul(fcdiv[:], fc[:], 1.0 / 64.0)
                fcsel = fcsingle.tile([128, 32], F32, tag="fcsel")
                nc.sync.dma_start(fcsel[0:64, :], fcdiv[:, 0:32])
                nc.sync.dma_start(fcsel[64:128, :], fcdiv[:, 32:64])

                covUT8 = fcsingle.tile([128, NCH, 64], F8, tag="covUT8")
                covUT16 = fcsingle.tile([128, NCH, 64], F16, tag="covUT16")
                fcT16 = fcsingle.tile([64, 64], F16, tag="fcT16")
                featT = fcsingle.tile([64, 64], F32, tag="featT")
                pwcsb = fcsingle.tile([64, OPC], F32, tag="pwcsb")
                thirdparts = fcsingle.tile([128, 8, OPC], F32, tag="thparts")

                # prefetch the folded-w3 stream in 8 medium DMAs, one tile per
                # DMA chunk (DMA-issue on SP costs ~650ns each, so not 58
                # small DMAs; >16KB/partition DMAs inside For_i wedge the
                # device, so not one big DMA; per-chunk tiles let rep i+1's
                # DMA overlap rep i's later matmuls)
                DMACH = 8
                w3tiles = []
                for t0 in range(0, NBLK, DMACH):
                    t1 = min(t0 + DMACH, NBLK)
                    w3ch = w3pool.tile([128, (t1 - t0) * 1024], F8,
                                       tag=f"w3c{t0}", name=f"w3c{t0}")
                    nc.sync.dma_start(w3ch[:], w3s[:, 1024 * t0:1024 * t1])
                    for t in range(t0, t1):
                        w3tiles.append(w3ch[:, 1024 * (t - t0):1024 * (t - t0 + 1)])

                # packed covU (pairs j<=k) via gather-matmuls, in transposed
                # K-major layout: covUT[p, kt, b] = fc[b, pj]*fc[b, pk]
                with tc.tile_pool(name="prepT", bufs=2, space="PSUM") as prepT, \
                     tc.tile_pool(name="prepA", bufs=2, space="PSUM") as prepA, \
                     tc.tile_pool(name="prepB", bufs=2, space="PSUM") as prepB:
                    pT = prepT.tile([128, 64], F32, tag="pT")
                    nc.tensor.transpose(pT[0:64, :], fc[:], ident[:])
                    nc.vector.tensor_scalar_mul(fcT16[:], pT[0:64, :], 8.0)
                    pT2 = prepT.tile([128, 64], F32, tag="pT")
                    nc.tensor.transpose(pT2[0:64, :], feat[:], ident[:])
                    nc.vector.tensor_copy(featT[:], pT2[0:64, :])
                    for q in range(5):
                        nch = min(4, NCH - 4 * q)
                        pA = prepA.tile([128, 4, 64], F32, tag="pA")
                        pB = prepB.tile([128, 4, 64], F32, tag="pB")
                        for c in range(nch):
                            ch = 4 * q + c
                            nc.tensor.matmul(pA[:, c, :],
                                             pjsb[:, 128 * ch:128 * (ch + 1)],
                                             fcT16[:], start=True, stop=True)
                            nc.tensor.matmul(pB[:, c, :],
                                             pksb[:, 128 * ch:128 * (ch + 1)],
                                             fcT16[:], start=True, stop=True)
                        # (two PSUM operands in one DVE op is rejected; copy
                        # the A side to SBUF via the scalar engine first)
                        sA = fcwork.tile([128, 4, 64], F16, tag="sA")
                        nc.scalar.activation(
                            sA[:, 0:nch, :].rearrange("p a b -> p (a b)"),
                            pA[:, 0:nch, :].rearrange("p a b -> p (a b)"),
                            AF.Copy)
                        nc.vector.tensor_mul(covUT16[:, 4 * q:4 * q + nch, :],
                                             sA[:, 0:nch, :], pB[:, 0:nch, :])
                        nc.vector.tensor_mul(covUT8[:, 4 * q:4 * q + nch, :],
                                             sA[:, 0:nch, :], pB[:, 0:nch, :])

                # wc (half-sym) + wm pre-pass
                with tc.tile_pool(name="wcp", bufs=1, space="PSUM") as wcp:
                    pwc = wcp.tile([64, OPC], F32)
                    for kt in range(NWCH):
                        nc.tensor.matmul(pwc[:], covUT16[:, kt, :], wcsb[:, kt, :],
                                         start=(kt == 0), stop=False)
                    nc.tensor.matmul(pwc[:], featT[:], wmsb[:], start=False,
                                     stop=True)
                    nc.vector.tensor_copy(pwcsb[:], pwc[:])

                # ---- DoubleRow block-sparse w3 stream ----
                with tc.tile_pool(name="drpsum", bufs=1, space="PSUM") as drpsum:
                    pb = []
                    for t in range(8):
                        pbt = drpsum.tile([128, 512], F32, tag=f"pb{t}",
                                          name=f"pb{t}")
                        pb.append(pbt)
                    idx = 0
                    for g in range(NG):
                        lhs = covUT8[:, 2 * g:2 * g + 2, :]
                        for b in range(NBG[g]):
                            t, hl = b % 8, b // 8
                            rhs2 = w3tiles[idx].rearrange(
                                "p (a n) -> p a n", a=2)
                            if hl == 0:
                                nc.tensor.matmul(
                                    pb[t][0:64, :], lhs, rhs2,
                                    start=(g == 0), stop=(g == GMAX[b]),
                                    perf_mode=mybir.MatmulPerfMode.DoubleRow,
                                    tile_position=(0, 0),
                                    skip_group_check=True,
                                )
                            else:
                                # DoubleRow + column tile_position 64 fails
                                # walrus ISA check; these 12 shallow blocks
                                # run as plain fp8 matmuls instead
                                for kt in range(2):
                                    nc.tensor.matmul(
                                        pb[t][64:128, :],
                                        covUT8[:, 2 * g + kt, :],
                                        rhs2[:, kt, :],
                                        start=(g == 0 and kt == 0),
                                        stop=(g == GMAX[b] and kt == 1),
                                        tile_position=(0, 64),
                                        skip_group_check=True,
                                    )
                            idx += 1

                    # contraction over i: third[b,o] += fc[b,i] * T1[b,(i,o)]
                    # tiles in psum-readiness order; 2 tiles routed via Pool
                    order = sorted(range(8), key=lambda t: GMAX[t])
                    pool_tiles = set(order[1:3])
                    for t in order:
                        if t in pool_tiles:
                            tmpf = fcwork.tile([128, 4, OPC], F16, tag="tmpf")
                            nc.scalar.activation(
                                tmpf[:].rearrange("p a o -> p (a o)"),
                                pb[t][:], AF.Copy)
                            tmm = fcwork.tile([128, 4, OPC], F16, tag="tmm")
                            nc.gpsimd.tensor_mul(
                                tmm[:], tmpf[:],
                                fcsel[:, 4 * t:4 * t + 4].unsqueeze(2)
                                .broadcast_to([128, 4, OPC]))
                            nc.vector.tensor_reduce(
                                thirdparts[:, t, :],
                                tmm[:].transpose([0, 2, 1]),
                                AX.X, op=ALU.add)
                        else:
                            tmpf = fcwork.tile([128, 4, OPC], F16, tag="tmpf")
                            for il in range(4):
                                nc.scalar.activation(
                                    tmpf[:, il, :],
                                    pb[t][:, il * OPC:(il + 1) * OPC],
                                    AF.Copy,
                                    scale=fcsel[:, 4 * t + il:4 * t + il + 1])
                            nc.vector.tensor_reduce(
                                thirdparts[:, t, :],
                                tmpf[:].transpose([0, 2, 1]),
                                AX.X, op=ALU.add)

                # ---- final assembly ----
                thsum = fcsingle.tile([128, OPC], F32, tag="thsum")
                nc.vector.tensor_reduce(
                    thsum[:], thirdparts[:].transpose([0, 2, 1]),
                    AX.X, op=ALU.add)
                thup = fcsingle.tile([64, OPC], F32, tag="thup")
                nc.sync.dma_start(thup[:], thsum[64:128, :])
                acc = fcsingle.tile([64, OPC], F32, tag="acc")
                nc.vector.tensor_add(acc[:], thsum[0:64, :], thup[:])
                nc.vector.tensor_add(acc[:], acc[:], pwcsb[:])
                nc.vector.tensor_add(acc[:], acc[:], bsum[:])
                nc.sync.dma_start(out[:], acc[:])

    nc.finalize()
    fixed, n_split = _split_multiwait_json(nc.to_json_bytes())
    nc.to_json_bytes = lambda: fixed
    return nc


_NC_CACHE = None


def _get_nc():
    global _NC_CACHE
    if _NC_CACHE is None:
        _NC_CACHE = _build()
    return _NC_CACHE


def _prepare_in_maps(inputs):
    x = np.asarray(inputs["x"])
    w1 = np.asarray(inputs["w1"])
    b1 = np.asarray(inputs["b1"])
    w2 = np.asarray(inputs["w2"])
    b2 = np.asarray(inputs["b2"])
    wm = np.asarray(inputs["wm"])
    bm = np.asarray(inputs["bm"])
    wc = np.asarray(inputs["wc"])
    bc = np.asarray(inputs["bc"])
    w3 = np.asarray(inputs["w3"])
    b3 = np.asarray(inputs["b3"])

    bf16 = ml_dtypes.bfloat16
    xb = x.astype(bf16)
    # pre-tiled conv1 input: xb3[core][q, r, (dy,ci), c, y, 1+w]
    #   = x[core*8 + 2c + r, ci, 2*(16q+y)+dy-1, w]  (zeros at pads)
    NQH = H1 // YC_HOST
    xb3 = np.zeros((NCORES, NQH, 2, 9, 4, YC_HOST, W + 2), dtype=bf16)
    rows = 2 * (np.arange(NQH * YC_HOST).reshape(NQH, YC_HOST))[None, None, :, :] \
        + np.arange(3).reshape(1, 3, 1, 1) - 1          # [1, dy, q, y]
    valid = (rows >= 0) & (rows < H)
    rowsc = np.clip(rows, 0, H - 1)
    # gather: xb3[n, q, r, dy*3+ci, c, y, 1:] = xb[n*8+2c+r, ci, rows[dy,q,y], :]
    for r in range(2):
        for c in range(4):
            img = xb[2 * c + r::BL, :, :, :]            # [NCORES, 3, H, W]
            g = img[:, :, rowsc[0], :]                  # [NCORES, 3, dy, q, y, W]
            g = g * valid[0][None, None, :, :, :, None].astype(bf16)
            xb3[:, :, r, :, c, :, 1:1 + W] = (
                g.transpose(0, 3, 2, 1, 4, 5).reshape(NCORES, NQH, 9, YC_HOST, W))
    w1t = np.ascontiguousarray(w1.transpose(2, 1, 3, 0)).reshape(9, 3, 32).astype(bf16)
    w2t = np.ascontiguousarray(w2.transpose(1, 2, 3, 0)).astype(bf16)
    b1r = np.ascontiguousarray(b1.reshape(32, 1)).astype(np.float32)
    b2r = np.ascontiguousarray(b2.reshape(64, 1)).astype(np.float32)

    # ---- symmetric fold of w3 ------------------------------------------
    fp8 = ml_dtypes.float8_e4m3
    OUT = NCORES * OPC
    w3r = w3.reshape(OUT, FEAT, FEAT, FEAT).astype(np.float32)
    s = w3r.copy()
    for perm in [(0, 1, 3, 2), (0, 2, 1, 3), (0, 2, 3, 1), (0, 3, 1, 2),
                 (0, 3, 2, 1)]:
        s += w3r.transpose(perm)
    ii, jj, kk = np.meshgrid(np.arange(FEAT), np.arange(FEAT),
                             np.arange(FEAT), indexing="ij")
    multc = 1.0 + (ii == jj) + (jj == kk) + 3 * ((ii == jj) & (jj == kk))
    s *= (1.0 / multc)[None]

    # static gather indices for the packed DoubleRow stream [128, TOTC]
    pair_arr = np.asarray(_PAIRS_P, np.int32)          # [2304, 2]
    real = np.arange(NPAIR_PAD) < NPAIR
    IDX = np.zeros((128, TOTC), np.int32)    # (i*64+j)*64+k monomial index
    OIDX = np.zeros((128, TOTC), np.int32)   # local output index o
    VALID = np.zeros((128, TOTC), bool)
    col = 0
    o_ = np.arange(OPC, dtype=np.int32)[None, None, None, :]
    for g in range(NG):
        seg = slice(g * 256, (g + 1) * 256)
        pjg = pair_arr[seg, 0].reshape(2, 128).T[:, :, None, None]  # [128,2,1,1]
        pkg = pair_arr[seg, 1].reshape(2, 128).T[:, :, None, None]
        rl = real[seg].reshape(2, 128).T[:, :, None, None]
        for b in range(NBG[g]):
            i_ = (4 * b + np.arange(4, dtype=np.int32))[None, None, :, None]
            v = (i_ <= pjg) & rl
            idx = (i_ * FEAT + pjg) * FEAT + pkg                    # [128,2,4,1]
            IDX[:, col:col + 1024] = np.broadcast_to(
                idx, (128, 2, 4, OPC)).reshape(128, 1024)
            OIDX[:, col:col + 1024] = np.broadcast_to(
                o_, (128, 2, 4, OPC)).reshape(128, 1024)
            VALID[:, col:col + 1024] = np.broadcast_to(
                v, (128, 2, 4, OPC)).reshape(128, 1024)
            col += 1024

    # wc half-sym fold: wchalf[o, j, k] (j<=k) = wc[o,j,k] + wc[o,k,j] offdiag
    wc_r = wc.reshape(OUT, FEAT, FEAT).astype(np.float32)
    wchalf = wc_r + wc_r.transpose(0, 2, 1)
    wchalf[:, np.arange(FEAT), np.arange(FEAT)] = \
        wc_r[:, np.arange(FEAT), np.arange(FEAT)]
    # wct layout [128 p, NWCH kt, OPC o]: pair m = kt*128+p
    wcj = pair_arr[:NWCH * 128, 0].reshape(NWCH, 128).T    # [128, kt]
    wck = pair_arr[:NWCH * 128, 1].reshape(NWCH, 128).T
    wcreal = real[:NWCH * 128].reshape(NWCH, 128).T        # [128, kt]

    # gather matrices (f16): pj/pk one-hot, zeroed on padding
    pjm = np.zeros((64, NPAIR_PAD), np.float16)
    pkm = np.zeros((64, NPAIR_PAD), np.float16)
    m_ = np.arange(NPAIR)
    pjm[pair_arr[:NPAIR, 0], m_] = 1.0
    pkm[pair_arr[:NPAIR, 1], m_] = 1.0

    in_maps = []
    for c in range(NCORES):
        osl = slice(OPC * c, OPC * (c + 1))
        base = (OPC * c + OIDX).astype(np.int64) * (FEAT ** 3) + IDX
        vals = s.reshape(-1)[base]
        vals[~VALID] = 0.0
        w3sc = vals.astype(fp8)
        wcc = wchalf[osl]                                   # [OPC, F, F]
        wctc = wcc[:, wcj, wck].transpose(1, 2, 0)          # [128, kt, OPC]
        # 1/64 undoes the 8x-scaled covU gathers (see kernel fcT16)
        wctc = (wctc * wcreal[:, :, None] / 64.0).astype(np.float16)
        in_maps.append({
            "xb3": np.ascontiguousarray(xb3[c]),
            "w1t": w1t,
            "b1": b1r,
            "w2t": w2t,
            "b2": b2r,
            "wmt": np.ascontiguousarray(wm[osl].T).astype(np.float32),
            "wct": np.ascontiguousarray(wctc),
            "w3s": np.ascontiguousarray(w3sc),
            "pjt": pjm,
            "pkt": pkm,
            "bias3": np.stack([bm[osl], bc[osl], b3[osl]]).astype(np.float32),
        })

    return in_maps


def kernel(**inputs):
    in_maps = _prepare_in_maps(inputs)
    from concourse.bass_utils import run_bass_kernel_spmd

    res = run_bass_kernel_spmd(_get_nc(), in_maps, core_ids=list(range(NCORES)))
    return np.concatenate([res.results[c]["out"] for c in range(NCORES)], axis=1)


if __name__ == "__main__":
    nc = _build()
    print("built OK; instructions:",
          sum(len(bb.instructions) for f in nc.m.functions for bb in f.blocks))
    if "compile" in sys.argv:
        import tempfile
        from concourse.bass_utils import compile_bass_kernel
        d = tempfile.mkdtemp()
        print("compiling in", d)
        print("NEFF:", compile_bass_kernel(nc, d))

